# revision 1
# baseline (speedup 1.0000x reference)
"""Trainium2 Bass kernel: 16-head self-attention block (B=8, N=1024, C=1024).

Data-parallel over batch: each of the 8 NeuronCores processes one batch
element end-to-end (QKV proj -> attention -> softmax -> out proj). No
collectives. Compute in bf16 (fp32 PSUM accumulation).

History: v17 ~385us -> v18 ~289us -> this (~272us). Design notes:
  - all big inputs host-cast to bf16 (10MB/core HBM traffic vs 20MB f32;
    the prologue is DMA-bound, so this moved the attention start a lot).
  - x chunk DMAs issue first, round-robin over the three DMA-capable
    queues (sync/scalar/gpsimd, ahead of the weights on gpsimd); x is
    transposed on the PE (identity matmul). HW dma_start_transpose was
    tried and is RACY for this shape (~27% of elements scrambled).
  - A.V col-tiled: head A (M=64) at array cols 0:63, head B at 64:127
    run CONCURRENTLY into one PSUM bank (rows 0:64 / 64:128); the odd
    head lands on partitions 64:128 directly (no stage_odd DMA).
  - softmax denominator via a second col-tiled pass with an all-ones
    lhsT: dn rows 0:64 = colsum(exp_A) broadcast across partitions,
    rows 64:128 = colsum(exp_B). Same 512-cycle stream cost the old
    v|ones trick paid, but the epilogue becomes Ln[128,512] +
    Exp(-x)[128,512] on ACT (ACT cost depends only on free dim -> one
    op covers both heads) + ONE DVE multiply. No PE work in the
    epilogue -> no per-segment PE drain -> HAM stays at K=8/8 2.4GHz
    (v17 oscillated 17x and paid ~50us at half clock).
  - software-pipelined emission: scores(km+2) is emitted before A.V(km);
    qkT/v/proj fills slot into the per-km ACT-pace slack; the previous
    segment's Ln/Exp/mul interleave into the next segment's first two
    score slots (ACT FIFO: EXP(7), Ln, EXP(0'), recip, EXP(1') ...).
  - PSUM budget (8 banks): spool 2x[128,1024] (4) + avp 2x[128,512] (2)
    + dnp 1 + mmp 1; prologue x-transposes borrow slots from all four.
  - proj for token blocks 0:512 runs inside pair 7's second half; the
    tail rotates its accumulator over three pools so chains never wait
    on the bias-add.
Steady state measured: fills at 216ns/MM (ideal), col-pairs 240ns,
scores boundaries +~150ns (LDW port). PE active ~248us of ~272us wall.
"""

import sys

sys.path.insert(0, "/opt/trn_rl_repo")

import numpy as np

P = 128
N = 1024  # tokens
C = 1024  # channels
H = 16  # heads
DH = 64  # head dim
NPAIR = 8  # head pairs
CO = C // P  # 8 outer chunks of contraction dim
NO = N // P  # 8 outer chunks of token dim
SCALE = DH ** -0.5
KERNEL_VERSION = 42  # bump on every semantic change (busts stale NEFF caches)

_CACHE = {}


def build_nc(dbg=False):
    import concourse.bass as bass
    import concourse.tile as tile
    from concourse import bacc, masks, mybir

    # Route Exp to natural_log_exp_and_others (which also holds Ln) so the
    # exp(-ln(s)) reciprocal shares one ACT table set with the softmax exp.
    if not getattr(bacc, "_exp_ln_patch", False):
        _orig_tables = bacc.get_activation_tables

        def _patched_tables(arch):
            t = _orig_tables(arch)
            for name, fns in t.items():
                if name != "natural_log_exp_and_others":
                    fns.discard(mybir.ActivationFunctionType.Exp)
            return t

        bacc.get_activation_tables = _patched_tables
        bacc._exp_ln_patch = True

    f32 = mybir.dt.float32
    bf16 = mybir.dt.bfloat16
    EXP = mybir.ActivationFunctionType.Exp
    LN = mybir.ActivationFunctionType.Ln

    nc = bacc.Bacc(None, target_bir_lowering=False)

    x_ext = nc.declare_dram_parameter("x", [N, C], bf16, isOutput=False)
    wqkv_ext = nc.declare_dram_parameter("qkv_w", [C, 3 * C], bf16, isOutput=False)
    wproj_ext = nc.declare_dram_parameter("proj_w", [C, C], bf16, isOutput=False)
    pb_ext = nc.declare_dram_parameter("proj_b", [C], f32, isOutput=False)
    out_ext = nc.declare_dram_parameter("out", [N, C], bf16, isOutput=True)
    # tiny version-stamped output: busts any executable cache keyed on the
    # HLO signature, and lets the harness confirm which kernel build ran
    ver_ext = nc.declare_dram_parameter(
        "kver", [1, KERNEL_VERSION], f32, isOutput=True
    )

    with tile.TileContext(nc) as tc:
        with (
            tc.tile_pool(name="big", bufs=1) as big,
            tc.tile_pool(name="work", bufs=3) as work,
            tc.tile_pool(name="ptp", bufs=6) as ptp,
            tc.tile_pool(name="mmp", bufs=1, space="PSUM") as mmp,
            tc.tile_pool(name="spool", bufs=2, space="PSUM") as spool,
            tc.tile_pool(name="avp", bufs=2, space="PSUM") as avp,
            tc.tile_pool(name="dnp", bufs=1, space="PSUM") as dnp,
        ):
            # ---------------- constants / big buffers ----------------
            wq = big.tile([P, CO, C], bf16, tag="wq")
            wk = big.tile([P, CO, C], bf16, tag="wk")
            wv = big.tile([P, CO, C], bf16, tag="wv")
            wproj = big.tile([P, CO, C], bf16, tag="wproj")
            pb = big.tile([P, C], f32, tag="pb")
            xTs = [
                big.tile([P, N], bf16, tag=f"xT{co}", name=f"xT{co}")
                for co in range(CO)
            ]
            xfs = [
                big.tile([P, C], bf16, tag=f"xf{no}", name=f"xf{no}")
                for no in range(NO)
            ]
            v_all = big.tile([P, NO, H, DH], bf16, tag="v_all")
            qT = big.tile([P, NPAIR, N], bf16, tag="qT")
            kT = big.tile([P, NPAIR, N], bf16, tag="kT")
            outT = big.tile([P, NPAIR, N], bf16, tag="outT")
            ident = big.tile([P, P], bf16, tag="ident")
            ones_t = big.tile([P, DH], bf16, tag="ones_t")
            ver_sb = big.tile([1, KERNEL_VERSION], f32, tag="ver_sb")

            # ---------------- input DMAs (issue everything early) -----
            # identity/ones first: they only need the gpsimd ALU, and the
            # first x transpose is gated on ident -- emitting them before
            # the dma_start descriptor generation saves ~7us of prologue.
            # (HW dma_start_transpose was tried and is RACY for this shape:
            # ~27% of elements land scrambled; PE transposes it is.)
            nc.vector.memset(ones_t, 1.0)
            masks.make_identity(nc, ident)
            nc.vector.memset(ver_sb, float(KERNEL_VERSION))

            # x chunks first, spread over all three DMA-capable queues
            # (sync/scalar/gpsimd) so they don't contend with the weight
            # stream; gpsimd's x chunks are enqueued ahead of the weights.
            x_q = [nc.sync, nc.scalar, nc.gpsimd]
            for no in range(NO):
                if no < 2:
                    # first chunks split in half: the co<4 transposes only
                    # need columns 0:512, so they start ~1.5us earlier
                    for h in range(2):
                        x_q[no % 3].dma_start(
                            out=xfs[no][:, h * 512 : (h + 1) * 512],
                            in_=x_ext[
                                no * P : (no + 1) * P, h * 512 : (h + 1) * 512
                            ],
                        )
                else:
                    x_q[no % 3].dma_start(
                        out=xfs[no], in_=x_ext[no * P : (no + 1) * P, :]
                    )
            # weights on the gpsimd queue; pair-0 q/k slices + v lo first
            wqkv_src = wqkv_ext[:, :].rearrange("(o p) j -> p o j", p=P)
            nc.gpsimd.dma_start(out=wq[:, :, 0:P], in_=wqkv_src[:, :, 0:P])
            nc.gpsimd.dma_start(
                out=wk[:, :, 0:P], in_=wqkv_src[:, :, C : C + P]
            )
            nc.gpsimd.dma_start(
                out=wv[:, :, 0:512], in_=wqkv_src[:, :, 2 * C : 2 * C + 512]
            )
            nc.gpsimd.dma_start(out=wq[:, :, P:C], in_=wqkv_src[:, :, P:C])
            nc.gpsimd.dma_start(
                out=wk[:, :, P:C], in_=wqkv_src[:, :, C + P : 2 * C]
            )
            nc.gpsimd.dma_start(
                out=wv[:, :, 512:1024],
                in_=wqkv_src[:, :, 2 * C + 512 : 3 * C],
            )
            pb_ap = pb_ext[:]
            pb_src = bass.AP(
                tensor=pb_ap.tensor,
                offset=pb_ap.offset,
                ap=[[0, P], pb_ap.ap[0]],
            )
            nc.gpsimd.dma_start(out=pb, in_=pb_src)

            # x transposes borrow the attention pools' PSUM slots
            # (prologue-only use); rotating over 4 tags keeps ~6 transposes
            # in flight so the DVE copy-out never gates the PE.
            tp_pools = [(spool, "S"), (avp, "av"), (dnp, "dn"), (mmp, "mm")]

            def x_transpose(no):
                for co in range(CO):
                    pool, tag = tp_pools[co % 4]
                    pst = pool.tile([P, P], bf16, tag=tag, name="pst")
                    nc.tensor.transpose(
                        pst, xfs[no][:, co * P : (co + 1) * P], ident
                    )
                    nc.vector.tensor_copy(
                        xTs[co][:, no * P : (no + 1) * P], pst
                    )

            # ---------------- helpers ----------------
            def qk_group(pair, which, nh):
                """One q^T/k^T half: 8 accumulating matmuls + copy-out."""
                w = wq if which == 0 else wk
                dst = qT if which == 0 else kT
                ps = mmp.tile([P, 512], f32, tag="mm", name="ps")
                for co in range(CO):
                    nc.tensor.matmul(
                        ps,
                        w[:, co, pair * P : (pair + 1) * P],
                        xTs[co][:, nh * 512 : (nh + 1) * 512],
                        start=(co == 0),
                        stop=(co == CO - 1),
                    )
                if which == 0:
                    # fold softmax scale into q
                    nc.vector.tensor_scalar_mul(
                        dst[:, pair, nh * 512 : (nh + 1) * 512], ps, SCALE
                    )
                else:
                    nc.vector.tensor_copy(
                        dst[:, pair, nh * 512 : (nh + 1) * 512], ps
                    )

            def v_half(no, jh):
                """v columns for heads jh*8..jh*8+8, token chunk no."""
                ps = mmp.tile([P, 512], f32, tag="mm", name="ps")
                for co in range(CO):
                    nc.tensor.matmul(
                        ps,
                        xTs[co][:, no * P : (no + 1) * P],
                        wv[:, co, jh * 512 : (jh + 1) * 512],
                        start=(co == 0),
                        stop=(co == CO - 1),
                    )
                nc.vector.tensor_copy(
                    v_all[:, no, jh * 8 : (jh + 1) * 8, :],
                    ps[:].rearrange("p (h d) -> p h d", h=8),
                )

            def proj_half(no, jh, pool_tag=None):
                """Output projection for token block no, channel half jh.

                pool_tag rotates the PSUM accumulator across otherwise-idle
                pools so back-to-back chains don't serialize on the single
                mmp buffer (the DVE bias-add holds it ~0.7us per chain).
                """
                pool, tag = pool_tag or (mmp, "mm")
                ps = pool.tile([P, 512], f32, tag=tag, name="ps")
                for pair in range(NPAIR):
                    nc.tensor.matmul(
                        ps,
                        outT[:, pair, no * P : (no + 1) * P],
                        wproj[:, pair, jh * 512 : (jh + 1) * 512],
                        start=(pair == 0),
                        stop=(pair == NPAIR - 1),
                    )
                res = work.tile([P, 512], bf16, tag="res", name="res")
                nc.vector.tensor_add(res, ps, pb[:, jh * 512 : (jh + 1) * 512])
                nc.sync.dma_start(
                    out=out_ext[no * P : (no + 1) * P, jh * 512 : (jh + 1) * 512],
                    in_=res,
                )

            # pending epilogue from the previous (pair, nh) segment:
            # (av, dn, pair, nsl); its ln/exp/mul are emitted interleaved
            # into the NEXT segment's first two score slots.
            pending = [None]

            # pairs 4-6 are ACT-bound (fills thin out, Scalar measures
            # ~95% busy): their reciprocal runs on the mostly-idle DVE
            # (stock iterative divide, ~4.3us for [128,512] f32) instead
            # of ACT's Ln+Exp; dn is staged to SBUF first so its PSUM
            # bank frees immediately.
            DVE_RECIP_PAIRS = ()

            def emit_ln():
                av_p, dn_p, pair_p, nsl_p = pending[0]
                if pair_p in DVE_RECIP_PAIRS:
                    dn_sb = work.tile([P, 512], f32, tag="dnsb", name="dn_sb")
                    nc.vector.tensor_copy(dn_sb, dn_p)
                    return dn_sb
                ln_t = work.tile([P, 512], f32, tag="ln_t", name="ln_t")
                nc.scalar.activation(ln_t, dn_p, LN)
                return ln_t

            def emit_recip_mul(ln_t):
                av_p, dn_p, pair_p, nsl_p = pending[0]
                if pair_p in DVE_RECIP_PAIRS:
                    rf = work.tile([P, 512], f32, tag="recf", name="rf")
                    nc.vector.reciprocal(rf, ln_t)
                    nc.vector.tensor_mul(outT[:, pair_p, nsl_p], av_p, rf)
                else:
                    rec = work.tile([P, 512], bf16, tag="rec", name="rec")
                    nc.scalar.activation(rec, ln_t, EXP, scale=-1.0)
                    nc.vector.tensor_mul(outT[:, pair_p, nsl_p], av_p, rec)
                pending[0] = None

            def segment(pair, nh, sf):
                hA, hB = 2 * pair, 2 * pair + 1
                nsl = slice(nh * 512, (nh + 1) * 512)
                av = avp.tile([P, 512], f32, tag="av", name="av")
                dn = dnp.tile([P, 512], f32, tag="dn", name="dn")
                pts = {}

                def scores(km):
                    s = spool.tile([P, N], f32, tag="S", name="s")
                    nc.tensor.matmul(
                        s[:, 0:512],
                        kT[0:DH, pair, km * P : (km + 1) * P],
                        qT[0:DH, pair, nsl],
                    )
                    nc.tensor.matmul(
                        s[:, 512:1024],
                        kT[DH:P, pair, km * P : (km + 1) * P],
                        qT[DH:P, pair, nsl],
                        tile_position=(DH, 0),
                    )
                    # exp (scores are O(1): no max subtraction needed)
                    pt = ptp.tile([P, N], bf16, tag="pt", name="pt")
                    nc.scalar.activation(pt, s, EXP)
                    pts[km] = pt

                scores(0)
                ln_t = emit_ln() if pending[0] else None
                scores(1)
                if ln_t is not None:
                    emit_recip_mul(ln_t)
                # In PE-bound pairs, adjacent exp tiles are summed on the
                # (mostly idle) DVE first so the denominator matmuls only
                # stream every OTHER km: -512 PE cycles per 2 km. Pairs 4-6
                # keep per-km dn (their DVE is busy with the reciprocal).
                pair_dn = pair not in DVE_RECIP_PAIRS
                pt_prev = None
                for km in range(NO):
                    if km + 2 < NO:
                        scores(km + 2)
                    for fn in sf.get((nh, km), ()):
                        fn()
                    pt = pts.pop(km)
                    st, sp = (km == 0), (km == NO - 1)
                    # A.V col-tiled: head A -> rows 0:64, head B -> 64:128
                    nc.tensor.matmul(
                        av[0:DH, :], v_all[:, km, hA, :], pt[:, 0:512],
                        start=st, stop=sp,
                    )
                    nc.tensor.matmul(
                        av[DH:P, :], v_all[:, km, hB, :], pt[:, 512:1024],
                        start=st, stop=sp,
                    )
                    # denominators, broadcast across partitions by the
                    # all-ones stationary operand
                    if not pair_dn:
                        nc.tensor.matmul(
                            dn[0:DH, :], ones_t, pt[:, 0:512],
                            start=st, stop=sp,
                        )
                        nc.tensor.matmul(
                            dn[DH:P, :], ones_t, pt[:, 512:1024],
                            start=st, stop=sp,
                        )
                    elif km % 2 == 0:
                        pt_prev = pt
                    else:
                        pts2 = ptp.tile(
                            [P, N], bf16, tag="pts", name="pts2", bufs=2
                        )
                        nc.vector.tensor_add(pts2, pt_prev, pt)
                        nc.tensor.matmul(
                            dn[0:DH, :], ones_t, pts2[:, 0:512],
                            start=(km == 1), stop=sp,
                        )
                        nc.tensor.matmul(
                            dn[DH:P, :], ones_t, pts2[:, 512:1024],
                            start=(km == 1), stop=sp,
                        )
                pending[0] = (av, dn, pair, nsl)

            # ---------------- schedule ----------------
            # n-half 0 of qT/kT only needs x chunks 0:4 -> start matmuls
            # while the remaining x chunks are still streaming in; the
            # last-needed chunk's transposes ride inside the qk group
            for no in range(4):
                x_transpose(no)
            qk_group(0, 0, 0)
            qk_group(0, 1, 0)
            for no in range(4, NO):
                x_transpose(no)
            qk_group(0, 1, 1)  # kT high half: attention(0) needs all km
            # (v columns stream in as just-in-time fills inside pair 0:
            # they depend on the wv DMA, and waiting for it here would
            # idle the PE right before attention and re-trip HAM.)

            def qkt_fill(pair):
                return [
                    lambda w=w, n=n: qk_group(pair, w, n)
                    for w in range(2)
                    for n in range(2)
                ]

            def make_fills(pair):
                sf = {}
                if pair == 0:
                    # v chunks just in time: v(km) lands right before its
                    # A.V; qT n-half 1 rides slot 2 (needed by nh 1)
                    for km in range(NO):
                        sf[(0, km)] = [lambda k=km: v_half(k, 0)]
                    sf[(0, 2)].append(lambda: qk_group(0, 0, 1))
                    q = qkt_fill(1)
                    for i, km in enumerate((0, 2, 4, 6)):
                        sf[(1, km)] = [q[i]]
                elif pair in (1, 2):
                    # heads 8-15 v columns (needed from pair 4) + next qkT
                    q = qkt_fill(pair + 1)
                    vs = [
                        lambda k=k: v_half(k, 1)
                        for k in range((pair - 1) * 4, pair * 4)
                    ]
                    sf[(0, 0)] = [q[0]]
                    sf[(0, 2)] = [vs[0]]
                    sf[(0, 4)] = [q[1]]
                    sf[(0, 6)] = [vs[1]]
                    sf[(1, 0)] = [q[2]]
                    sf[(1, 2)] = [vs[2]]
                    sf[(1, 4)] = [q[3]]
                    sf[(1, 6)] = [vs[3]]
                elif pair < NPAIR - 2:
                    q = qkt_fill(pair + 1)
                    for i, s in enumerate(((0, 1), (0, 5), (1, 1), (1, 5))):
                        sf[s] = [q[i]]
                elif pair == NPAIR - 2:
                    # pair 7's qT n-half 1 is only needed by its nh 1 --
                    # it rides inside pair-7-nh0 (which has no other fills
                    # and idles ~3us ACT-bound); the rest stays here
                    q = qkt_fill(7)
                    sf[(0, 1)] = [q[0]]
                    sf[(0, 5)] = [q[2]]
                    sf[(1, 1)] = [q[3]]
                else:
                    sf[(0, 2)] = [lambda: qk_group(7, 0, 1)]
                    # pair 7: token blocks 0:512 of the projection can run
                    # as soon as nh 0's epilogue lands (mmp only: the other
                    # PSUM pools are still owned by this segment's av/dn)
                    for i, (no, jh) in enumerate(
                        (n, j) for n in range(4) for j in range(2)
                    ):
                        sf[(1, i)] = [lambda n=no, j=jh: proj_half(n, j)]
                return sf

            for pair in range(NPAIR):
                if pair == 3:
                    # proj weights only needed at the tail; load mid-flight
                    nc.gpsimd.dma_start(
                        out=wproj,
                        in_=wproj_ext[:, :].rearrange("(o p) j -> p o j", p=P),
                    )
                sf = make_fills(pair)
                for nh in range(2):
                    segment(pair, nh, sf)

            # flush the final epilogue (pair 7, nh 1)
            emit_recip_mul(emit_ln())
            nc.sync.dma_start(out=ver_ext[:, :], in_=ver_sb)

            # ---------------- output projection tail ----------------
            # all attention pools are idle now; rotate the accumulator so
            # consecutive chains never wait on the bias-add
            rot = [(mmp, "mm"), (avp, "av"), (dnp, "dn")]
            i = 0
            for no in range(4, NO):
                for jh in range(2):
                    proj_half(no, jh, rot[i % 3])
                    i += 1

    nc.compile()
    return nc


def _get_nc():
    if "nc" not in _CACHE:
        _CACHE["nc"] = build_nc()
    return _CACHE["nc"]


def make_in_maps(inputs):
    """Per-core input dicts: batch elem i -> core i, big tensors in bf16."""
    import ml_dtypes

    bf16 = ml_dtypes.bfloat16
    x = np.asarray(inputs["x"]).astype(bf16)
    qkv_w = np.asarray(inputs["qkv_w"]).astype(bf16)
    proj_w = np.asarray(inputs["proj_w"]).astype(bf16)
    proj_b = np.asarray(inputs["proj_b"], dtype=np.float32)
    B = x.shape[0]
    assert B == 8, f"kernel hardcoded for B=8, got {B}"
    return [
        {"x": x[i], "qkv_w": qkv_w, "proj_w": proj_w, "proj_b": proj_b}
        for i in range(B)
    ]


def kernel(**inputs) -> np.ndarray:
    """Full-input entry point: shards batch over 8 cores, returns [8,N,C]."""
    from concourse.bass_utils import run_bass_kernel_spmd

    in_maps = make_in_maps(inputs)
    nc = _get_nc()
    res = run_bass_kernel_spmd(nc, in_maps, core_ids=list(range(8)))
    out = np.stack([res.results[i]["out"] for i in range(8)], axis=0)
    return out.astype(np.float32)



# revision 3
# speedup vs baseline: 1.0057x; 1.0057x over previous
"""Trainium2 Bass kernel: 16-head self-attention block (B=8, N=1024, C=1024).

Data-parallel over batch: each of the 8 NeuronCores processes one batch
element end-to-end (QKV proj -> attention -> softmax -> out proj). No
collectives. Compute in bf16 (fp32 PSUM accumulation).

History: v17 ~385us -> v18 ~289us -> v19 ~262us -> this (v20).
v20 redesign, driven by the trace + cost model:
  - the PE p-state ramp: any PE idle gap drops the clock to 1.2GHz for
    the next 3us of work. v19's ACT-paced stretches (1 fill/km) showed
    PE slices stretched 216->330ns. Fix: a static waterfill spreads the
    fill matmuls (qkv/v/proj groups, chopped into 2-MM chunks) over
    EVERY km slot so the PE is always oversubscribed (~1.1-1.4us/slot
    vs ACT's 1.0us EXP pace) and never gaps.
  - mmp (fill accumulator) now 2 PSUM bufs: back-to-back fill groups no
    longer stall on the previous group's DVE copy-out. The bank comes
    from avp (2->1): av is staged to SBUF by DVE right after the last
    A.V, freeing its bank for the next segment.
  - softmax epilogue moved off ACT: 1/dn via DVE reciprocal_approx_fast
    (~2^-18 rel err, one op) + DVE multiply. ACT now runs EXPs only --
    a clean 128 x 1026ns chain with no Ln/Exp(-1) hiccups at segment
    boundaries (v19 paid ~1.3us ACT per boundary).
  - dn (softmax denominators) kept from v19: pairwise DVE pre-add of
    exp tiles, ones-stationary col-tiled matmuls every other km.
  - fills at 216ns/MM, scores/AV col-pairs ~216ns when PE stays at
    2.4GHz; PE streaming work totals ~181us -> wall target ~200us.
"""

import sys

sys.path.insert(0, "/opt/trn_rl_repo")

import numpy as np

P = 128
N = 1024  # tokens
C = 1024  # channels
H = 16  # heads
DH = 64  # head dim
NPAIR = 8  # head pairs
CO = C // P  # 8 outer chunks of contraction dim
NO = N // P  # 8 outer chunks of token dim
NSEG = 2 * NPAIR  # 16 segments, pair-major: seg = 2*pair + nh
SCALE = DH ** -0.5
KERNEL_VERSION = 50  # bump on every semantic change (busts stale NEFF caches)

_CACHE = {}


def build_nc(dbg=False):
    import concourse.bass as bass
    import concourse.tile as tile
    from concourse import bacc, masks, mybir

    f32 = mybir.dt.float32
    bf16 = mybir.dt.bfloat16
    EXP = mybir.ActivationFunctionType.Exp

    nc = bacc.Bacc(None, target_bir_lowering=False)

    x_ext = nc.declare_dram_parameter("x", [N, C], bf16, isOutput=False)
    wqkv_ext = nc.declare_dram_parameter("qkv_w", [C, 3 * C], bf16, isOutput=False)
    wproj_ext = nc.declare_dram_parameter("proj_w", [C, C], bf16, isOutput=False)
    pb_ext = nc.declare_dram_parameter("proj_b", [C], f32, isOutput=False)
    out_ext = nc.declare_dram_parameter("out", [N, C], bf16, isOutput=True)
    # tiny version-stamped output: busts any executable cache keyed on the
    # HLO signature, and lets the harness confirm which kernel build ran
    ver_ext = nc.declare_dram_parameter(
        "kver", [1, KERNEL_VERSION], f32, isOutput=True
    )

    with tile.TileContext(nc) as tc:
        with (
            tc.tile_pool(name="big", bufs=1) as big,
            tc.tile_pool(name="work", bufs=3) as work,
            tc.tile_pool(name="avsp", bufs=2) as avsp,
            tc.tile_pool(name="ptp", bufs=6) as ptp,
            tc.tile_pool(name="mmp", bufs=2, space="PSUM") as mmp,
            tc.tile_pool(name="spool", bufs=2, space="PSUM") as spool,
            tc.tile_pool(name="avp", bufs=1, space="PSUM") as avp,
            tc.tile_pool(name="dnp", bufs=1, space="PSUM") as dnp,
        ):
            # ---------------- constants / big buffers ----------------
            wq = big.tile([P, CO, C], bf16, tag="wq")
            wk = big.tile([P, CO, C], bf16, tag="wk")
            wv = big.tile([P, CO, C], bf16, tag="wv")
            wproj = big.tile([P, CO, C], bf16, tag="wproj")
            pb = big.tile([P, C], f32, tag="pb")
            xTs = [
                big.tile([P, N], bf16, tag=f"xT{co}", name=f"xT{co}")
                for co in range(CO)
            ]
            xfs = [
                big.tile([P, C], bf16, tag=f"xf{no}", name=f"xf{no}")
                for no in range(NO)
            ]
            v_all = big.tile([P, NO, H, DH], bf16, tag="v_all")
            qT = big.tile([P, NPAIR, N], bf16, tag="qT")
            kT = big.tile([P, NPAIR, N], bf16, tag="kT")
            outT = big.tile([P, NPAIR, N], bf16, tag="outT")
            ident = big.tile([P, P], bf16, tag="ident")
            ones_t = big.tile([P, DH], bf16, tag="ones_t")
            ver_sb = big.tile([1, KERNEL_VERSION], f32, tag="ver_sb")

            # ---------------- input DMAs (issue everything early) -----
            # identity/ones first: they only need the gpsimd ALU, and the
            # first x transpose is gated on ident -- emitting them before
            # the dma_start descriptor generation saves ~7us of prologue.
            # (HW dma_start_transpose was tried and is RACY for this shape:
            # ~27% of elements land scrambled; PE transposes it is.)
            nc.vector.memset(ones_t, 1.0)
            masks.make_identity(nc, ident)
            nc.vector.memset(ver_sb, float(KERNEL_VERSION))

            # x chunks first, spread over all three DMA-capable queues
            # (sync/scalar/gpsimd) so they don't contend with the weight
            # stream; gpsimd's x chunks are enqueued ahead of the weights.
            x_q = [nc.sync, nc.scalar, nc.gpsimd]
            for no in range(NO):
                if no < 2:
                    # first chunks split in half: the co<4 transposes only
                    # need columns 0:512, so they start ~1.5us earlier
                    for h in range(2):
                        x_q[no % 3].dma_start(
                            out=xfs[no][:, h * 512 : (h + 1) * 512],
                            in_=x_ext[
                                no * P : (no + 1) * P, h * 512 : (h + 1) * 512
                            ],
                        )
                else:
                    x_q[no % 3].dma_start(
                        out=xfs[no], in_=x_ext[no * P : (no + 1) * P, :]
                    )
            # weights on the gpsimd queue; pair-0 q/k slices + v lo first
            wqkv_src = wqkv_ext[:, :].rearrange("(o p) j -> p o j", p=P)
            nc.gpsimd.dma_start(out=wq[:, :, 0:P], in_=wqkv_src[:, :, 0:P])
            nc.gpsimd.dma_start(
                out=wk[:, :, 0:P], in_=wqkv_src[:, :, C : C + P]
            )
            nc.gpsimd.dma_start(
                out=wv[:, :, 0:512], in_=wqkv_src[:, :, 2 * C : 2 * C + 512]
            )
            nc.gpsimd.dma_start(out=wq[:, :, P:C], in_=wqkv_src[:, :, P:C])
            nc.gpsimd.dma_start(
                out=wk[:, :, P:C], in_=wqkv_src[:, :, C + P : 2 * C]
            )
            nc.gpsimd.dma_start(
                out=wv[:, :, 512:1024],
                in_=wqkv_src[:, :, 2 * C + 512 : 3 * C],
            )
            pb_ap = pb_ext[:]
            pb_src = bass.AP(
                tensor=pb_ap.tensor,
                offset=pb_ap.offset,
                ap=[[0, P], pb_ap.ap[0]],
            )
            nc.gpsimd.dma_start(out=pb, in_=pb_src)

            # x transposes borrow the attention pools' PSUM slots
            # (prologue-only use); rotating over 4 tags keeps ~6 transposes
            # in flight so the DVE copy-out never gates the PE.
            tp_pools = [(spool, "S"), (avp, "av"), (dnp, "dn"), (mmp, "mm")]

            def x_transpose(no):
                for co in range(CO):
                    pool, tag = tp_pools[co % 4]
                    pst = pool.tile([P, P], bf16, tag=tag, name="pst")
                    nc.tensor.transpose(
                        pst, xfs[no][:, co * P : (co + 1) * P], ident
                    )
                    nc.vector.tensor_copy(
                        xTs[co][:, no * P : (no + 1) * P], pst
                    )

            # ---------------- fill groups, chunked ----------------
            # Each fill group is 8 accumulating matmuls + a DVE copy-out,
            # emitted as 4 chunks of 2 MMs so the waterfill can spread them
            # across km slots. mmp bufs=2 lets group G+1's first chunk run
            # while group G's copy-out drains on DVE.

            def qk_chunks(pair, which, nh):
                """q^T/k^T half for one pair: -> list of (n_mms, emit_fn)."""
                w = wq if which == 0 else wk
                dst = qT if which == 0 else kT
                st = {}

                def mk(ci):
                    def f():
                        if ci == 0:
                            st["ps"] = mmp.tile(
                                [P, 512], f32, tag="mm", name="ps"
                            )
                        ps = st["ps"]
                        for co in (2 * ci, 2 * ci + 1):
                            nc.tensor.matmul(
                                ps,
                                w[:, co, pair * P : (pair + 1) * P],
                                xTs[co][:, nh * 512 : (nh + 1) * 512],
                                start=(co == 0),
                                stop=(co == CO - 1),
                            )

                    return f

                def copyout():
                    ps = st["ps"]
                    if which == 0:
                        # fold softmax scale into q
                        nc.vector.tensor_scalar_mul(
                            dst[:, pair, nh * 512 : (nh + 1) * 512], ps, SCALE
                        )
                    else:
                        nc.vector.tensor_copy(
                            dst[:, pair, nh * 512 : (nh + 1) * 512], ps
                        )

                return [(2, mk(ci)) for ci in range(4)] + [(0, copyout)]

            def v_chunks(no, jh):
                """v columns for heads jh*8..jh*8+8, token chunk no."""
                st = {}

                def mk(ci):
                    def f():
                        if ci == 0:
                            st["ps"] = mmp.tile(
                                [P, 512], f32, tag="mm", name="ps"
                            )
                        ps = st["ps"]
                        for co in (2 * ci, 2 * ci + 1):
                            nc.tensor.matmul(
                                ps,
                                xTs[co][:, no * P : (no + 1) * P],
                                wv[:, co, jh * 512 : (jh + 1) * 512],
                                start=(co == 0),
                                stop=(co == CO - 1),
                            )

                    return f

                def copyout():
                    nc.vector.tensor_copy(
                        v_all[:, no, jh * 8 : (jh + 1) * 8, :],
                        st["ps"][:].rearrange("p (h d) -> p h d", h=8),
                    )

                return [(2, mk(ci)) for ci in range(4)] + [(0, copyout)]

            def proj_chunks(no, jh):
                """Output projection for token block no, channel half jh."""
                st = {}

                def mk(ci):
                    def f():
                        if ci == 0:
                            st["ps"] = mmp.tile(
                                [P, 512], f32, tag="mm", name="ps"
                            )
                        ps = st["ps"]
                        for pair in (2 * ci, 2 * ci + 1):
                            nc.tensor.matmul(
                                ps,
                                outT[:, pair, no * P : (no + 1) * P],
                                wproj[:, pair, jh * 512 : (jh + 1) * 512],
                                start=(pair == 0),
                                stop=(pair == NPAIR - 1),
                            )

                    return f

                def copyout():
                    res = work.tile([P, 512], bf16, tag="res", name="res")
                    nc.vector.tensor_add(
                        res, st["ps"], pb[:, jh * 512 : (jh + 1) * 512]
                    )
                    oq = [nc.sync, nc.gpsimd][(no * 2 + jh) % 2]
                    oq.dma_start(
                        out=out_ext[
                            no * P : (no + 1) * P, jh * 512 : (jh + 1) * 512
                        ],
                        in_=res,
                    )

                return [(2, mk(ci)) for ci in range(4)] + [(0, copyout)]

            # ---------------- static fill schedule (waterfill) --------
            # Feeder items in dependency order, each with a deadline in
            # global slot units (seg*8+km; the item must be emitted before
            # that slot's mandatory work). Placement targets uniform PE
            # oversubscription: never drain the queue early (back half
            # starvation = p-state crash), never miss a deadline.
            def slot_of(seg, km):
                return seg * 8 + km

            feeder = []  # (n_mms, emit_fn, deadline_slot)

            def add_group(chunks, deadline_slot):
                for n, f in chunks:
                    feeder.append((n, f, deadline_slot))

            # qT pair0 n-half 1: needed by seg 1
            add_group(qk_chunks(0, 0, 1), slot_of(1, 0))
            for pr in range(1, NPAIR):
                # kT both halves + qT n0 by seg 2p; qT n1 by seg 2p+1
                add_group(qk_chunks(pr, 1, 0), slot_of(2 * pr, 0))
                add_group(qk_chunks(pr, 1, 1), slot_of(2 * pr, 0))
                add_group(qk_chunks(pr, 0, 0), slot_of(2 * pr, 0))
                add_group(qk_chunks(pr, 0, 1), slot_of(2 * pr + 1, 0))
                if pr == 4:
                    # v hi half (heads 8-15) lands here in feeder ORDER so
                    # its chunks trickle through segs 5-7; deadline is the
                    # consuming km of pair 4 (seg 8). Pairs 5-7 consume the
                    # same tiles later.
                    for no in range(NO):
                        add_group(v_chunks(no, 1), slot_of(8, no))

            # proj token blocks 0:4 (outT nh0 cols): available only after
            # pair 7 nh0's epilogue lands (early in seg 15) -- placed in
            # seg 15 by the avail bound below, deadline = none (tail).
            TAIL_SLOT = slot_of(NSEG, 0)
            proj_items = []
            for no in range(4):
                for jh in range(2):
                    proj_items.append(proj_chunks(no, jh))

            # Build per-slot assignment: slots (0,0)..(15,7).
            # Pinned: pair0 nh0 slot km gets v(km, lo) just-in-time.
            # Feeder: waterfill with per-slot cap, rate = max(deadline
            # pressure, uniform remainder rate) so the queue lasts to the
            # end of seg 14. proj fills seg 15 at cap; rest goes to tail.
            assign = {(s, k): [] for s in range(NSEG) for k in range(8)}
            for km in range(NO):
                assign[(0, km)].extend(f for _, f in v_chunks(km, 0))

            idx = 0  # feeder cursor
            NSLOTS = slot_of(15, 0)  # feeder must drain by seg 15
            for s in range(NSLOTS):
                seg, km = divmod(s, 8)
                if idx >= len(feeder):
                    break
                # deadline pressure: mms that must go out by each future
                # deadline, divided by slots remaining until it
                need = 0.0
                acc = 0
                for n, _f, dl in feeder[idx:]:
                    acc += n
                    if dl <= s:
                        need = max(need, float(acc) + 99.0)  # overdue: flush
                    elif dl < TAIL_SLOT:
                        need = max(need, acc / (dl - s))
                rem = sum(n for n, _f, _dl in feeder[idx:])
                uniform = rem / (NSLOTS - s)
                target = max(need, uniform)
                cap = 6 if seg > 0 else 4
                take = 0
                while idx < len(feeder) and (
                    take < target or feeder[idx][0] == 0
                ):
                    n, f, _dl = feeder[idx]
                    if take + n > cap and n > 0 and take >= target:
                        break
                    assign[(seg, km)].append(f)
                    take += n
                    idx += 1
            assert idx >= len(feeder), (
                f"feeder not drained by seg 15: {len(feeder) - idx} left"
            )

            # proj nh0 into seg 15 slots: 8 groups over 8 slots
            for i, chunks in enumerate(proj_items):
                assign[(15, i)].extend(f for _, f in chunks)

            # ---------------- attention ----------------
            # pending epilogue from the previous segment:
            # (av_sb, dn, pair, nsl); recip+mul are emitted interleaved
            # into the NEXT segment's first two score slots (both DVE).
            pending = [None]

            def emit_recip():
                _av_sb, dn_p, _pair_p, _nsl_p = pending[0]
                rf = work.tile([P, 512], f32, tag="rf", name="rf")
                nc.vector.reciprocal_approx_fast(out=rf, in_=dn_p)
                return rf

            def emit_mul(rf):
                av_sb, _dn_p, pair_p, nsl_p = pending[0]
                nc.vector.tensor_mul(outT[:, pair_p, nsl_p], av_sb, rf)
                pending[0] = None

            def segment(seg):
                pair, nh = divmod(seg, 2)
                hA, hB = 2 * pair, 2 * pair + 1
                nsl = slice(nh * 512, (nh + 1) * 512)
                av = avp.tile([P, 512], f32, tag="av", name="av")
                dn = dnp.tile([P, 512], f32, tag="dn", name="dn")
                pts = {}

                def scores(km):
                    s = spool.tile([P, N], f32, tag="S", name="s")
                    nc.tensor.matmul(
                        s[:, 0:512],
                        kT[0:DH, pair, km * P : (km + 1) * P],
                        qT[0:DH, pair, nsl],
                    )
                    nc.tensor.matmul(
                        s[:, 512:1024],
                        kT[DH:P, pair, km * P : (km + 1) * P],
                        qT[DH:P, pair, nsl],
                        tile_position=(DH, 0),
                    )
                    # exp (scores are O(1): no max subtraction needed)
                    pt = ptp.tile([P, N], bf16, tag="pt", name="pt")
                    nc.scalar.activation(pt, s, EXP)
                    pts[km] = pt

                scores(0)
                rf = emit_recip() if pending[0] else None
                scores(1)
                if rf is not None:
                    emit_mul(rf)
                # Adjacent exp tiles are summed on the DVE first so the
                # denominator matmuls only stream every OTHER km: -512 PE
                # cycles per 2 km.
                pt_prev = None
                for km in range(NO):
                    if km + 2 < NO:
                        scores(km + 2)
                    for fn in assign.get((seg, km), ()):
                        fn()
                    pt = pts.pop(km)
                    st, sp = (km == 0), (km == NO - 1)
                    # A.V col-tiled: head A -> rows 0:64, head B -> 64:128
                    nc.tensor.matmul(
                        av[0:DH, :], v_all[:, km, hA, :], pt[:, 0:512],
                        start=st, stop=sp,
                    )
                    nc.tensor.matmul(
                        av[DH:P, :], v_all[:, km, hB, :], pt[:, 512:1024],
                        start=st, stop=sp,
                    )
                    # denominators, broadcast across partitions by the
                    # all-ones stationary operand
                    if km % 2 == 0:
                        pt_prev = pt
                    else:
                        pts2 = ptp.tile(
                            [P, N], bf16, tag="pts", name="pts2", bufs=2
                        )
                        nc.vector.tensor_add(pts2, pt_prev, pt)
                        nc.tensor.matmul(
                            dn[0:DH, :], ones_t, pts2[:, 0:512],
                            start=(km == 1), stop=sp,
                        )
                        nc.tensor.matmul(
                            dn[DH:P, :], ones_t, pts2[:, 512:1024],
                            start=(km == 1), stop=sp,
                        )
                # stage av to SBUF: frees the single avp PSUM bank for the
                # next segment's accumulation before the epilogue runs
                av_sb = avsp.tile([P, 512], f32, tag="avst", name="av_sb")
                nc.vector.tensor_copy(av_sb, av)
                pending[0] = (av_sb, dn, pair, nsl)

            # ---------------- schedule ----------------
            # n-half 0 of qT/kT only needs x chunks 0:4 -> start matmuls
            # while the remaining x chunks are still streaming in
            for no in range(4):
                x_transpose(no)

            def run_group(chunks):
                for _n, f in chunks:
                    f()

            run_group(qk_chunks(0, 0, 0))
            run_group(qk_chunks(0, 1, 0))
            for no in range(4, NO):
                x_transpose(no)
            run_group(qk_chunks(0, 1, 1))  # kT high half: seg 0 needs all km

            for seg in range(NSEG):
                if seg == 6:
                    # proj weights only needed at the tail; load mid-flight
                    nc.gpsimd.dma_start(
                        out=wproj,
                        in_=wproj_ext[:, :].rearrange("(o p) j -> p o j", p=P),
                    )
                segment(seg)

            # flush the final epilogue (pair 7, nh 1)
            emit_mul(emit_recip())
            nc.sync.dma_start(out=ver_ext[:, :], in_=ver_sb)

            # ---------------- output projection tail ----------------
            # mmp's two bufs alternate so consecutive chains overlap the
            # bias-add + DMA of the previous one
            for no in range(4, NO):
                for jh in range(2):
                    run_group(proj_chunks(no, jh))

    nc.compile()
    return nc


def _get_nc():
    if "nc" not in _CACHE:
        _CACHE["nc"] = build_nc()
    return _CACHE["nc"]


def make_in_maps(inputs):
    """Per-core input dicts: batch elem i -> core i, big tensors in bf16."""
    import ml_dtypes

    bf16 = ml_dtypes.bfloat16
    x = np.asarray(inputs["x"]).astype(bf16)
    qkv_w = np.asarray(inputs["qkv_w"]).astype(bf16)
    proj_w = np.asarray(inputs["proj_w"]).astype(bf16)
    proj_b = np.asarray(inputs["proj_b"], dtype=np.float32)
    B = x.shape[0]
    assert B == 8, f"kernel hardcoded for B=8, got {B}"
    return [
        {"x": x[i], "qkv_w": qkv_w, "proj_w": proj_w, "proj_b": proj_b}
        for i in range(B)
    ]


def kernel(**inputs) -> np.ndarray:
    """Full-input entry point: shards batch over 8 cores, returns [8,N,C]."""
    from concourse.bass_utils import run_bass_kernel_spmd

    in_maps = make_in_maps(inputs)
    nc = _get_nc()
    res = run_bass_kernel_spmd(nc, in_maps, core_ids=list(range(8)))
    out = np.stack([res.results[i]["out"] for i in range(8)], axis=0)
    return out.astype(np.float32)


# revision 9
# speedup vs baseline: 1.0106x; 1.0049x over previous
"""Trainium2 Bass kernel: 16-head self-attention block (B=8, N=1024, C=1024).

Data-parallel over batch: each of the 8 NeuronCores processes one batch
element end-to-end (QKV proj -> attention -> softmax -> out proj). No
collectives. Compute in bf16 (fp32 PSUM accumulation).

History: v17 ~385us -> v18 ~289us -> v19 ~262us -> this (v20).
v20 redesign, driven by the trace + cost model:
  - the PE p-state ramp: any PE idle gap drops the clock to 1.2GHz for
    the next 3us of work. v19's ACT-paced stretches (1 fill/km) showed
    PE slices stretched 216->330ns. Fix: a static waterfill spreads the
    fill matmuls (qkv/v/proj groups, chopped into 2-MM chunks) over
    EVERY km slot so the PE is always oversubscribed (~1.1-1.4us/slot
    vs ACT's 1.0us EXP pace) and never gaps.
  - mmp (fill accumulator) now 2 PSUM bufs: back-to-back fill groups no
    longer stall on the previous group's DVE copy-out. The bank comes
    from avp (2->1): av is staged to SBUF by DVE right after the last
    A.V, freeing its bank for the next segment.
  - softmax epilogue moved off ACT: 1/dn via DVE reciprocal_approx_fast
    (~2^-18 rel err, one op) + DVE multiply. ACT now runs EXPs only --
    a clean 128 x 1026ns chain with no Ln/Exp(-1) hiccups at segment
    boundaries (v19 paid ~1.3us ACT per boundary).
  - dn (softmax denominators) kept from v19: pairwise DVE pre-add of
    exp tiles, ones-stationary col-tiled matmuls every other km.
  - fills at 216ns/MM, scores/AV col-pairs ~216ns when PE stays at
    2.4GHz; PE streaming work totals ~181us -> wall target ~200us.
"""

import sys

sys.path.insert(0, "/opt/trn_rl_repo")

import numpy as np

P = 128
N = 1024  # tokens
C = 1024  # channels
H = 16  # heads
DH = 64  # head dim
NPAIR = 8  # head pairs
CO = C // P  # 8 outer chunks of contraction dim
NO = N // P  # 8 outer chunks of token dim
NSEG = 2 * NPAIR  # 16 segments, pair-major: seg = 2*pair + nh
SCALE = DH ** -0.5
KERNEL_VERSION = 51  # bump on every semantic change (busts stale NEFF caches)

_CACHE = {}


def build_nc(dbg=False):
    import concourse.bass as bass
    import concourse.tile as tile
    from concourse import bacc, masks, mybir

    f32 = mybir.dt.float32
    bf16 = mybir.dt.bfloat16
    EXP = mybir.ActivationFunctionType.Exp

    nc = bacc.Bacc(None, target_bir_lowering=False)

    x_ext = nc.declare_dram_parameter("x", [N, C], bf16, isOutput=False)
    wqkv_ext = nc.declare_dram_parameter("qkv_w", [C, 3 * C], bf16, isOutput=False)
    wproj_ext = nc.declare_dram_parameter("proj_w", [C, C], bf16, isOutput=False)
    pb_ext = nc.declare_dram_parameter("proj_b", [C], f32, isOutput=False)
    out_ext = nc.declare_dram_parameter("out", [N, C], bf16, isOutput=True)
    # tiny version-stamped output: busts any executable cache keyed on the
    # HLO signature, and lets the harness confirm which kernel build ran
    ver_ext = nc.declare_dram_parameter(
        "kver", [1, KERNEL_VERSION], f32, isOutput=True
    )

    with tile.TileContext(nc) as tc:
        with (
            tc.tile_pool(name="big", bufs=1) as big,
            tc.tile_pool(name="work", bufs=3) as work,
            tc.tile_pool(name="avsp", bufs=2) as avsp,
            tc.tile_pool(name="ptp", bufs=6) as ptp,
            tc.tile_pool(name="mmp", bufs=2, space="PSUM") as mmp,
            tc.tile_pool(name="spool", bufs=2, space="PSUM") as spool,
            tc.tile_pool(name="avp", bufs=1, space="PSUM") as avp,
            tc.tile_pool(name="dnp", bufs=1, space="PSUM") as dnp,
        ):
            # ---------------- constants / big buffers ----------------
            wq = big.tile([P, CO, C], bf16, tag="wq")
            wk = big.tile([P, CO, C], bf16, tag="wk")
            wv = big.tile([P, CO, C], bf16, tag="wv")
            wproj = big.tile([P, CO, C], bf16, tag="wproj")
            pb = big.tile([P, C], f32, tag="pb")
            xTs = [
                big.tile([P, N], bf16, tag=f"xT{co}", name=f"xT{co}")
                for co in range(CO)
            ]
            xfs = [
                big.tile([P, C], bf16, tag=f"xf{no}", name=f"xf{no}")
                for no in range(NO)
            ]
            v_all = big.tile([P, NO, H, DH], bf16, tag="v_all")
            qT = big.tile([P, NPAIR, N], bf16, tag="qT")
            kT = big.tile([P, NPAIR, N], bf16, tag="kT")
            outT = big.tile([P, NPAIR, N], bf16, tag="outT")
            ident = big.tile([P, P], bf16, tag="ident")
            ones_t = big.tile([P, DH], bf16, tag="ones_t")
            ver_sb = big.tile([1, KERNEL_VERSION], f32, tag="ver_sb")

            # ---------------- input DMAs (issue everything early) -----
            # identity/ones first: they only need the gpsimd ALU, and the
            # first x transpose is gated on ident -- emitting them before
            # the dma_start descriptor generation saves ~7us of prologue.
            # (HW dma_start_transpose was tried and is RACY for this shape:
            # ~27% of elements land scrambled; PE transposes it is.)
            nc.vector.memset(ones_t, 1.0)
            masks.make_identity(nc, ident)
            nc.vector.memset(ver_sb, float(KERNEL_VERSION))

            # x chunks first, spread over all three DMA-capable queues
            # (sync/scalar/gpsimd) so they don't contend with the weight
            # stream; gpsimd's x chunks are enqueued ahead of the weights.
            x_q = [nc.sync, nc.scalar, nc.gpsimd]
            for no in range(NO):
                if no < 2:
                    # first chunks split in half: the co<4 transposes only
                    # need columns 0:512, so they start ~1.5us earlier
                    for h in range(2):
                        x_q[no % 3].dma_start(
                            out=xfs[no][:, h * 512 : (h + 1) * 512],
                            in_=x_ext[
                                no * P : (no + 1) * P, h * 512 : (h + 1) * 512
                            ],
                        )
                else:
                    x_q[no % 3].dma_start(
                        out=xfs[no], in_=x_ext[no * P : (no + 1) * P, :]
                    )
            # weights on the gpsimd queue; pair-0 q/k slices + v lo first
            wqkv_src = wqkv_ext[:, :].rearrange("(o p) j -> p o j", p=P)
            nc.gpsimd.dma_start(out=wq[:, :, 0:P], in_=wqkv_src[:, :, 0:P])
            nc.gpsimd.dma_start(
                out=wk[:, :, 0:P], in_=wqkv_src[:, :, C : C + P]
            )
            nc.gpsimd.dma_start(
                out=wv[:, :, 0:512], in_=wqkv_src[:, :, 2 * C : 2 * C + 512]
            )
            nc.gpsimd.dma_start(out=wq[:, :, P:C], in_=wqkv_src[:, :, P:C])
            nc.gpsimd.dma_start(
                out=wk[:, :, P:C], in_=wqkv_src[:, :, C + P : 2 * C]
            )
            nc.gpsimd.dma_start(
                out=wv[:, :, 512:1024],
                in_=wqkv_src[:, :, 2 * C + 512 : 3 * C],
            )
            pb_ap = pb_ext[:]
            pb_src = bass.AP(
                tensor=pb_ap.tensor,
                offset=pb_ap.offset,
                ap=[[0, P], pb_ap.ap[0]],
            )
            nc.gpsimd.dma_start(out=pb, in_=pb_src)

            # x transposes borrow the attention pools' PSUM slots
            # (prologue-only use); rotating over 4 tags keeps ~6 transposes
            # in flight so the DVE copy-out never gates the PE.
            tp_pools = [(spool, "S"), (avp, "av"), (dnp, "dn"), (mmp, "mm")]

            def x_transpose(no):
                for co in range(CO):
                    pool, tag = tp_pools[co % 4]
                    pst = pool.tile([P, P], bf16, tag=tag, name="pst")
                    nc.tensor.transpose(
                        pst, xfs[no][:, co * P : (co + 1) * P], ident
                    )
                    nc.vector.tensor_copy(
                        xTs[co][:, no * P : (no + 1) * P], pst
                    )

            # ---------------- fill groups, chunked ----------------
            # Each fill group is 8 accumulating matmuls + a DVE copy-out,
            # emitted as 4 chunks of 2 MMs so the waterfill can spread them
            # across km slots. mmp bufs=2 lets group G+1's first chunk run
            # while group G's copy-out drains on DVE.

            def qk_chunks(pair, which, nh):
                """q^T/k^T half for one pair: -> list of (n_mms, emit_fn)."""
                w = wq if which == 0 else wk
                dst = qT if which == 0 else kT
                st = {}

                def mk(ci):
                    def f():
                        if ci == 0:
                            st["ps"] = mmp.tile(
                                [P, 512], f32, tag="mm", name="ps"
                            )
                        ps = st["ps"]
                        for co in (2 * ci, 2 * ci + 1):
                            nc.tensor.matmul(
                                ps,
                                w[:, co, pair * P : (pair + 1) * P],
                                xTs[co][:, nh * 512 : (nh + 1) * 512],
                                start=(co == 0),
                                stop=(co == CO - 1),
                            )

                    return f

                def copyout():
                    ps = st["ps"]
                    if which == 0:
                        # fold softmax scale into q
                        nc.vector.tensor_scalar_mul(
                            dst[:, pair, nh * 512 : (nh + 1) * 512], ps, SCALE
                        )
                    else:
                        nc.vector.tensor_copy(
                            dst[:, pair, nh * 512 : (nh + 1) * 512], ps
                        )

                return [(2, mk(ci)) for ci in range(4)] + [(0, copyout)]

            def v_chunks(no, jh):
                """v columns for heads jh*8..jh*8+8, token chunk no."""
                st = {}

                def mk(ci):
                    def f():
                        if ci == 0:
                            st["ps"] = mmp.tile(
                                [P, 512], f32, tag="mm", name="ps"
                            )
                        ps = st["ps"]
                        for co in (2 * ci, 2 * ci + 1):
                            nc.tensor.matmul(
                                ps,
                                xTs[co][:, no * P : (no + 1) * P],
                                wv[:, co, jh * 512 : (jh + 1) * 512],
                                start=(co == 0),
                                stop=(co == CO - 1),
                            )

                    return f

                def copyout():
                    nc.vector.tensor_copy(
                        v_all[:, no, jh * 8 : (jh + 1) * 8, :],
                        st["ps"][:].rearrange("p (h d) -> p h d", h=8),
                    )

                return [(2, mk(ci)) for ci in range(4)] + [(0, copyout)]

            def proj_chunks(no, jh):
                """Output projection for token block no, channel half jh."""
                st = {}

                def mk(ci):
                    def f():
                        if ci == 0:
                            st["ps"] = mmp.tile(
                                [P, 512], f32, tag="mm", name="ps"
                            )
                        ps = st["ps"]
                        for pair in (2 * ci, 2 * ci + 1):
                            nc.tensor.matmul(
                                ps,
                                outT[:, pair, no * P : (no + 1) * P],
                                wproj[:, pair, jh * 512 : (jh + 1) * 512],
                                start=(pair == 0),
                                stop=(pair == NPAIR - 1),
                            )

                    return f

                def copyout():
                    res = work.tile([P, 512], bf16, tag="res", name="res")
                    nc.vector.tensor_add(
                        res, st["ps"], pb[:, jh * 512 : (jh + 1) * 512]
                    )
                    oq = [nc.sync, nc.gpsimd][(no * 2 + jh) % 2]
                    oq.dma_start(
                        out=out_ext[
                            no * P : (no + 1) * P, jh * 512 : (jh + 1) * 512
                        ],
                        in_=res,
                    )

                return [(2, mk(ci)) for ci in range(4)] + [(0, copyout)]

            # ---------------- static fill schedule (waterfill) --------
            # Feeder items in dependency order, each with a deadline in
            # global slot units (seg*8+km; the item must be emitted before
            # that slot's mandatory work). Placement targets uniform PE
            # oversubscription: never drain the queue early (back half
            # starvation = p-state crash), never miss a deadline.
            def slot_of(seg, km):
                return seg * 8 + km

            # nh-major segment order: segs 0-7 = pairs 0-7 nh0, segs 8-15
            # = pairs 0-7 nh1. proj for token rows 0:512 then unlocks at
            # seg 8 (all nh0 epilogues done) and feeds the whole back half
            # -- exactly where pair-major starved the PE.
            SEG_ORDER = [(p, 0) for p in range(NPAIR)] + [
                (p, 1) for p in range(NPAIR)
            ]

            feeder = []  # (n_mms, emit_fn, avail_slot, deadline_slot)
            TAIL_SLOT = slot_of(NSEG, 0)

            def add_group(chunks, deadline_slot, avail_slot=0):
                for n, f in chunks:
                    feeder.append((n, f, avail_slot, deadline_slot))

            for pr in range(1, NPAIR):
                # kT n0 (km 0-3) + full qT n0 by seg p slot 0; kT n1
                # (km 4-7) first read by scores(4), emitted in slot 2
                add_group(qk_chunks(pr, 1, 0), slot_of(pr, 0))
                add_group(qk_chunks(pr, 0, 0), slot_of(pr, 0))
                add_group(qk_chunks(pr, 1, 1), slot_of(pr, 2))
                if pr == 2:
                    # v hi half (heads 8-15): deadline = consuming km of
                    # pair 4 nh0 (seg 4); trickles through segs 2-3
                    for no in range(NO):
                        add_group(v_chunks(no, 1), slot_of(4, no))
            # qT n1 halves: needed by seg 8+p; proj nh0 groups (avail at
            # seg 8 after the last nh0 epilogue) interleave between them
            # so the back half stays uniformly fed through seg 15.
            add_group(qk_chunks(0, 0, 1), slot_of(8, 0))
            for pr in range(1, NPAIR):
                add_group(qk_chunks(pr, 0, 1), slot_of(8 + pr, 0))
                no, jh = (pr - 1) // 2, (pr - 1) % 2
                add_group(
                    proj_chunks(no, jh), TAIL_SLOT, avail_slot=slot_of(8, 0)
                )
            # remaining proj nh0 group (no deadline: uniform rate
            # spreads it across the rest of the back half)
            add_group(
                proj_chunks(3, 1), TAIL_SLOT, avail_slot=slot_of(8, 0)
            )

            # Build per-slot assignment: slots (0,0)..(15,7).
            # Pinned: pair0 nh0 slot km gets v(km, lo) just-in-time.
            # Feeder: waterfill, rate = max(deadline pressure, uniform
            # remainder rate) so the queue lasts to the end of seg 15.
            assign = {(s, k): [] for s in range(NSEG) for k in range(8)}
            for km in range(NO):
                assign[(0, km)].extend(f for _, f in v_chunks(km, 0))

            idx = 0  # feeder cursor
            NSLOTS = TAIL_SLOT
            for s in range(NSLOTS):
                seg, km = divmod(s, 8)
                if idx >= len(feeder):
                    break
                # deadline pressure: mms that must go out by each future
                # deadline, divided by slots remaining until it
                need = 0.0
                acc = 0
                for n, _f, _av, dl in feeder[idx:]:
                    acc += n
                    if dl <= s:
                        need = max(need, float(acc) + 99.0)  # overdue: flush
                    elif dl < TAIL_SLOT:
                        need = max(need, acc / (dl - s))
                rem = sum(n for n, _f, _av, _dl in feeder[idx:])
                uniform = rem / (NSLOTS - s)
                target = max(need, uniform)
                cap = 6 if seg > 0 else 4
                take = 0
                while idx < len(feeder) and (
                    take < target or feeder[idx][0] == 0
                ):
                    n, f, av, _dl = feeder[idx]
                    if av > s:
                        break
                    if take + n > cap and n > 0 and take >= target:
                        break
                    assign[(seg, km)].append(f)
                    take += n
                    idx += 1
            assert idx >= len(feeder), (
                f"feeder not drained by seg 15: {len(feeder) - idx} left"
            )

            # ---------------- attention ----------------
            # pending epilogue from the previous segment:
            # (av_sb, dn, pair, nsl); recip+mul are emitted interleaved
            # into the NEXT segment's first two score slots (both DVE).
            pending = [None]

            def emit_recip():
                _av_sb, dn_p, _pair_p, _nsl_p = pending[0]
                rf = work.tile([P, 512], f32, tag="rf", name="rf")
                nc.vector.reciprocal_approx_fast(out=rf, in_=dn_p)
                return rf

            def emit_mul(rf):
                av_sb, _dn_p, pair_p, nsl_p = pending[0]
                nc.vector.tensor_mul(outT[:, pair_p, nsl_p], av_sb, rf)
                pending[0] = None

            def segment(seg):
                pair, nh = SEG_ORDER[seg]
                hA, hB = 2 * pair, 2 * pair + 1
                nsl = slice(nh * 512, (nh + 1) * 512)
                av = avp.tile([P, 512], f32, tag="av", name="av")
                dn = dnp.tile([P, 512], f32, tag="dn", name="dn")
                pts = {}

                def scores(km):
                    s = spool.tile([P, N], f32, tag="S", name="s")
                    nc.tensor.matmul(
                        s[:, 0:512],
                        kT[0:DH, pair, km * P : (km + 1) * P],
                        qT[0:DH, pair, nsl],
                    )
                    nc.tensor.matmul(
                        s[:, 512:1024],
                        kT[DH:P, pair, km * P : (km + 1) * P],
                        qT[DH:P, pair, nsl],
                        tile_position=(DH, 0),
                    )
                    # exp (scores are O(1): no max subtraction needed)
                    pt = ptp.tile([P, N], bf16, tag="pt", name="pt")
                    nc.scalar.activation(pt, s, EXP)
                    pts[km] = pt

                scores(0)
                rf = emit_recip() if pending[0] else None
                scores(1)
                if rf is not None:
                    emit_mul(rf)
                # Adjacent exp tiles are summed on the DVE first so the
                # denominator matmuls only stream every OTHER km: -512 PE
                # cycles per 2 km.
                pt_prev = None
                for km in range(NO):
                    # fills first: they cover the tail of EXP(km) + sem
                    # propagation so scores(km+2)'s spool wait is hidden
                    for fn in assign.get((seg, km), ()):
                        fn()
                    if km + 2 < NO:
                        scores(km + 2)
                    pt = pts.pop(km)
                    st, sp = (km == 0), (km == NO - 1)
                    # A.V col-tiled: head A -> rows 0:64, head B -> 64:128
                    nc.tensor.matmul(
                        av[0:DH, :], v_all[:, km, hA, :], pt[:, 0:512],
                        start=st, stop=sp,
                    )
                    nc.tensor.matmul(
                        av[DH:P, :], v_all[:, km, hB, :], pt[:, 512:1024],
                        start=st, stop=sp,
                    )
                    # denominators, broadcast across partitions by the
                    # all-ones stationary operand
                    if km % 2 == 0:
                        pt_prev = pt
                    else:
                        pts2 = ptp.tile(
                            [P, N], bf16, tag="pts", name="pts2", bufs=2
                        )
                        nc.vector.tensor_add(pts2, pt_prev, pt)
                        nc.tensor.matmul(
                            dn[0:DH, :], ones_t, pts2[:, 0:512],
                            start=(km == 1), stop=sp,
                        )
                        nc.tensor.matmul(
                            dn[DH:P, :], ones_t, pts2[:, 512:1024],
                            start=(km == 1), stop=sp,
                        )
                # stage av to SBUF: frees the single avp PSUM bank for the
                # next segment's accumulation before the epilogue runs
                av_sb = avsp.tile([P, 512], f32, tag="avst", name="av_sb")
                nc.vector.tensor_copy(av_sb, av)
                pending[0] = (av_sb, dn, pair, nsl)

            # ---------------- schedule ----------------
            # n-half 0 of qT/kT only needs x chunks 0:4 -> start matmuls
            # while the remaining x chunks are still streaming in
            for no in range(4):
                x_transpose(no)

            def run_group(chunks):
                for _n, f in chunks:
                    f()

            run_group(qk_chunks(0, 0, 0))
            run_group(qk_chunks(0, 1, 0))
            for no in range(4, NO):
                x_transpose(no)
            run_group(qk_chunks(0, 1, 1))  # kT high half: seg 0 needs all km

            for seg in range(NSEG):
                if seg == 3:
                    # proj weights needed from seg 8 (first proj fills);
                    # load mid-flight once the input stream has drained
                    nc.gpsimd.dma_start(
                        out=wproj,
                        in_=wproj_ext[:, :].rearrange("(o p) j -> p o j", p=P),
                    )
                segment(seg)

            # flush the final epilogue (pair 7, nh 1)
            emit_mul(emit_recip())
            nc.sync.dma_start(out=ver_ext[:, :], in_=ver_sb)

            # ---------------- output projection tail ----------------
            # mmp's two bufs alternate so consecutive chains overlap the
            # bias-add + DMA of the previous one
            for no in range(4, NO):
                for jh in range(2):
                    run_group(proj_chunks(no, jh))

    nc.compile()
    return nc


def _get_nc():
    if "nc" not in _CACHE:
        _CACHE["nc"] = build_nc()
    return _CACHE["nc"]


def make_in_maps(inputs):
    """Per-core input dicts: batch elem i -> core i, big tensors in bf16."""
    import ml_dtypes

    bf16 = ml_dtypes.bfloat16
    x = np.asarray(inputs["x"]).astype(bf16)
    qkv_w = np.asarray(inputs["qkv_w"]).astype(bf16)
    proj_w = np.asarray(inputs["proj_w"]).astype(bf16)
    proj_b = np.asarray(inputs["proj_b"], dtype=np.float32)
    B = x.shape[0]
    assert B == 8, f"kernel hardcoded for B=8, got {B}"
    return [
        {"x": x[i], "qkv_w": qkv_w, "proj_w": proj_w, "proj_b": proj_b}
        for i in range(B)
    ]


def kernel(**inputs) -> np.ndarray:
    """Full-input entry point: shards batch over 8 cores, returns [8,N,C]."""
    from concourse.bass_utils import run_bass_kernel_spmd

    in_maps = make_in_maps(inputs)
    nc = _get_nc()
    res = run_bass_kernel_spmd(nc, in_maps, core_ids=list(range(8)))
    out = np.stack([res.results[i]["out"] for i in range(8)], axis=0)
    return out.astype(np.float32)
